# revision 12
# baseline (speedup 1.0000x reference)
"""Trainium2 Bass kernel for Bahdanau-style additive attention.

Reference computation (eval mode):
    query  = hidden @ Wq + bq                        # [B, OUT]
    h      = tanh(key + query[:, None, :])           # [B, T, OUT]
    scores = (h @ W1 + b1) @ W2 + b2                 # [B, T, 1] -> [B, T]
    scores = where(pos < seq_lens, scores, -inf)
    attn   = softmax(scores, axis=-1)                # [B, T]
    context= einsum('bt,btd->bd', attn, value)       # [B, DV]
    return (context, attn)

Algebraic simplifications (exact, weight-only folds):
  * No nonlinearity between W1 and W2 (dropout is identity in eval), so
    scores = tanh(...) @ (W1 @ W2) + (b1 @ W2 + b2).  w = W1@W2 is a single
    [OUT] vector, folded on the host (1M-FLOP weight constant-fold).
  * The scalar c = b1@W2 + b2 shifts all scores equally; softmax is
    shift-invariant, so c (and with it b1, b2) drops out entirely.
  * Softmax runs without max-subtraction: |scores| <= sum|w| ~ 11, so
    exp() is far from fp32 overflow/underflow.  Masking multiplies the
    exp by a 0/1 mask (exact zeros, like exp(-inf)).

Sharding: pure data-parallel over batch. B=32 across 8 cores -> 4 batches
per core; small weights replicated; no collectives.

Per-core dataflow (layout: t on partitions, d on free axis):
  pass 1 (scores), per batch, per [128, 4, 1024] key block (f32->bf16
  cast during SWDGE DMA):
    DVE  : h = key + q_bcast            (tensor_tensor add, bf16 2x)
    ACT  : th = tanh(h)
    DVE  : scores[:, i] = sum_d th*w    (tensor_tensor_reduce, fused)
  softmax on the [128, 16] score tile (t = 128*i + p):
    ACT exp -> DVE mask-mult (iota vs seq_len) -> DVE row-sum ->
    PE partition-sum (lhsT=rowsums, rhs=ones) -> DVE reciprocal ->
    PE K=1 broadcast matmul -> scale.
  pass 2 (context): PE matmuls lhsT=exp[:, i] [128,1], rhs=value block
    (bf16 cast-DMA), PSUM-accumulated over 16 t-chunks, scaled by 1/s.
"""

import numpy as np

B, T, HID, OUT, DV = 32, 2048, 1024, 1024, 1024
NCORES = 8
BL = B // NCORES          # 4 local batches per core
NT = T // 128             # 16 t-chunks per batch
TCH = 4                   # t-chunks per DMA block
NBLK = NT // TCH          # 4 blocks per batch

_CACHE = {}


def _build_nc():
    import concourse.bass as bass  # noqa: F401
    import concourse.bacc as bacc
    import concourse.tile as tile
    from concourse import mybir

    f32 = mybir.dt.float32
    bf16 = mybir.dt.bfloat16
    Alu = mybir.AluOpType
    Act = mybir.ActivationFunctionType

    nc = bacc.Bacc("TRN2")

    key_d = nc.declare_dram_parameter("key", [BL, T, OUT], f32, isOutput=False)
    val_d = nc.declare_dram_parameter("value", [BL, T, DV], f32, isOutput=False)
    hT_d = nc.declare_dram_parameter("hiddenT", [HID, BL], f32, isOutput=False)
    wq_d = nc.declare_dram_parameter("Wq", [HID, OUT], f32, isOutput=False)
    bq_d = nc.declare_dram_parameter("bq", [1, OUT], f32, isOutput=False)
    w_d = nc.declare_dram_parameter("w_fold", [1, OUT], f32, isOutput=False)
    seq_d = nc.declare_dram_parameter("seq_f32", [1, BL], f32, isOutput=False)
    iota_d = nc.declare_dram_parameter("iota_f32", [128, NT], f32, isOutput=False)
    ctx_o = nc.declare_dram_parameter("ctx_out", [BL, DV], f32, isOutput=True)
    attn_o = nc.declare_dram_parameter("attn_out", [BL, T], f32, isOutput=True)

    with tile.TileContext(nc) as tc:
        with (
            tc.tile_pool(name="const", bufs=1) as const,
            tc.tile_pool(name="dramp", bufs=1, space="DRAM") as dramp,
            tc.tile_pool(name="qbp", bufs=2) as qbp,
            tc.tile_pool(name="keyp", bufs=5) as keyp,
            tc.tile_pool(name="thp", bufs=3) as thp,
            tc.tile_pool(name="valp", bufs=5) as valp,
            tc.tile_pool(name="scorep", bufs=2) as scorep,
            tc.tile_pool(name="smallp", bufs=4) as smallp,
            tc.tile_pool(name="outp", bufs=2) as outp,
            tc.tile_pool(name="cps", bufs=2, space="PSUM") as cps,
            tc.tile_pool(name="sps", bufs=1, space="PSUM") as sps,
        ):
            # ---------------- setup ----------------
            hT_sb = const.tile([128, 8, BL], f32)
            nc.sync.dma_start(
                out=hT_sb, in_=hT_d[:].rearrange("(c p) b -> p c b", p=128)
            )
            bq_sb = const.tile([1, OUT], f32)
            nc.sync.dma_start(out=bq_sb, in_=bq_d[:])
            iota_f = const.tile([128, NT], f32)
            nc.sync.dma_start(out=iota_f, in_=iota_d[:])
            ones14 = const.tile([1, BL], f32)
            nc.vector.memset(ones14, 1.0)
            ones_col = const.tile([128, 1], f32)
            nc.vector.memset(ones_col, 1.0)
            ones_row = const.tile([1, 128], f32)
            nc.vector.memset(ones_row, 1.0)

            # w broadcast to 128 partitions, cast to bf16 during DMA
            w_bc = const.tile([128, OUT], bf16)
            nc.gpsimd.dma_start(out=w_bc, in_=w_d[0:1, :].to_broadcast((128, OUT)))

            wq_sb = const.tile([128, 8, OUT], f32)
            nc.sync.dma_start(
                out=wq_sb, in_=wq_d[:].rearrange("(c p) d -> p c d", p=128)
            )

            # query projection: q[b, d] = sum_h hidden[b, h] Wq[h, d] + bq[d]
            # (PSUM tiles share the context-pass pool tags; temporally disjoint)
            q_ps = [
                cps.tile([BL, 512], f32, name=f"qps{i}", tag=f"cps{i}")
                for i in range(2)
            ]
            for half in range(2):
                cols = slice(512 * half, 512 * (half + 1))
                for c in range(8):
                    nc.tensor.matmul(
                        out=q_ps[half],
                        lhsT=hT_sb[:, c, :],
                        rhs=wq_sb[:, c, cols],
                        start=(c == 0),
                        stop=False,
                    )
                nc.tensor.matmul(
                    out=q_ps[half],
                    lhsT=ones14,
                    rhs=bq_sb[:, cols],
                    start=False,
                    stop=True,
                )
            q_sb = const.tile([BL, OUT], bf16)
            for half in range(2):
                cols = slice(512 * half, 512 * (half + 1))
                nc.vector.tensor_copy(out=q_sb[:, cols], in_=q_ps[half])

            # round-trip q through DRAM so it can be partition-broadcast by DMA
            q_dram = dramp.tile([BL, OUT], bf16)
            nc.sync.dma_start(out=q_dram, in_=q_sb)

            # ---------------- per-batch main loop ----------------
            for b in range(BL):
                qb = qbp.tile([128, OUT], bf16)
                nc.sync.dma_start(
                    out=qb, in_=q_dram[b : b + 1, :].to_broadcast((128, OUT))
                )
                len_b = smallp.tile([128, 1], f32)
                nc.sync.dma_start(
                    out=len_b, in_=seq_d[0:1, b : b + 1].to_broadcast((128, 1))
                )

                scores = scorep.tile([128, NT], f32)
                key_b = key_d[b].rearrange("(i p) d -> p i d", p=128)  # [128,16,1024]

                for blk in range(NBLK):
                    isl = slice(TCH * blk, TCH * (blk + 1))
                    # pre-fill the tile with the query broadcast, then let the
                    # DMA accumulate key onto it (cast f32->bf16 + add in the
                    # SDMA datapath) -> no DVE add pass at all
                    kt = keyp.tile([128, TCH, OUT], bf16)
                    nc.vector.tensor_copy(
                        out=kt,
                        in_=qb.rearrange("p (u d) -> p u d", u=1).to_broadcast(
                            (128, TCH, OUT)
                        ),
                    )
                    nc.gpsimd.dma_start(
                        out=kt, in_=key_b[:, isl, :], accum_op=Alu.add
                    )
                    th = thp.tile([128, TCH, OUT], bf16)
                    nc.scalar.activation(out=th, in_=kt, func=Act.Tanh)
                    for j in range(TCH):
                        i = TCH * blk + j
                        nc.vector.scalar_tensor_tensor(
                            out=kt[:, j, :],  # scratch (kt is dead after tanh)
                            in0=th[:, j, :],
                            scalar=1.0,
                            in1=w_bc,
                            op0=Alu.mult,
                            op1=Alu.mult,
                            accum_out=scores[:, i : i + 1],
                        )

                # ---- softmax over the [128, 16] score tile (t = 128i + p) ----
                e32 = smallp.tile([128, NT], f32)
                nc.scalar.activation(out=e32, in_=scores, func=Act.Exp)
                mask01 = smallp.tile([128, NT], f32)
                nc.vector.tensor_scalar(
                    out=mask01,
                    in0=iota_f,
                    scalar1=len_b,
                    scalar2=None,
                    op0=Alu.is_lt,
                )
                em = smallp.tile([128, NT], f32)
                nc.vector.tensor_tensor(out=em, in0=e32, in1=mask01, op=Alu.mult)
                ebf = smallp.tile([128, NT], bf16)
                nc.vector.tensor_copy(out=ebf, in_=em)
                s1 = smallp.tile([128, 1], f32)
                nc.vector.reduce_sum(out=s1, in_=em, axis=mybir.AxisListType.X)
                sum_ps = sps.tile([1, 1], f32, name="sum_ps", tag="sum_ps")
                nc.tensor.matmul(out=sum_ps, lhsT=s1, rhs=ones_col)
                r1 = smallp.tile([1, 1], f32)
                nc.vector.reciprocal(out=r1, in_=sum_ps)
                r_ps = sps.tile([128, 1], f32, name="r_ps", tag="r_ps")
                nc.tensor.matmul(out=r_ps, lhsT=ones_row, rhs=r1)
                r_sb = smallp.tile([128, 1], f32)
                nc.vector.tensor_copy(out=r_sb, in_=r_ps)

                # attn output
                attn_sb = outp.tile([128, NT], f32)
                nc.vector.tensor_scalar(
                    out=attn_sb, in0=em, scalar1=r_sb, scalar2=None, op0=Alu.mult
                )
                nc.sync.dma_start(
                    out=attn_o[b].rearrange("(i p) -> p i", p=128), in_=attn_sb
                )

                # ---- context: ctx[d] = (sum_t em[t] * value[t, d]) / s ----
                c_ps = [
                    cps.tile([1, 512], f32, name=f"cps{i}", tag=f"cps{i}")
                    for i in range(2)
                ]
                val_b = val_d[b].rearrange("(i p) d -> p i d", p=128)
                for blk in range(NBLK):
                    isl = slice(TCH * blk, TCH * (blk + 1))
                    vt = valp.tile([128, TCH, DV], bf16)
                    nc.gpsimd.dma_start(out=vt, in_=val_b[:, isl, :])
                    for j in range(TCH):
                        i = TCH * blk + j
                        for half in range(2):
                            cols = slice(512 * half, 512 * (half + 1))
                            nc.tensor.matmul(
                                out=c_ps[half],
                                lhsT=ebf[:, i : i + 1],
                                rhs=vt[:, j, cols],
                                start=(i == 0),
                                stop=(i == NT - 1),
                            )
                ctx_sb = outp.tile([1, DV], f32)
                for half in range(2):
                    cols = slice(512 * half, 512 * (half + 1))
                    nc.vector.tensor_scalar(
                        out=ctx_sb[:, cols],
                        in0=c_ps[half],
                        scalar1=r_sb[0:1],
                        scalar2=None,
                        op0=Alu.mult,
                    )
                nc.sync.dma_start(out=ctx_o[b : b + 1, :], in_=ctx_sb)

    nc.compile()
    return nc


def get_nc():
    if "nc" not in _CACHE:
        _CACHE["nc"] = _build_nc()
    return _CACHE["nc"]


_IOTA = (
    (np.arange(NT, dtype=np.float32)[None, :] * 128)
    + np.arange(128, dtype=np.float32)[:, None]
)


def make_in_maps(hidden, key, value, seq_lens, Wq, bq, W1, b1, W2, b2):
    f32 = np.float32
    # weight-only constant fold; b1/b2 drop out of softmax (shift invariance)
    w_fold = (np.asarray(W1, f32) @ np.asarray(W2, f32)).reshape(1, OUT)
    iota = np.ascontiguousarray(_IOTA, dtype=f32)
    in_maps = []
    for c in range(NCORES):
        sl = slice(BL * c, BL * (c + 1))
        in_maps.append(
            {
                "key": np.ascontiguousarray(key[sl], dtype=f32),
                "value": np.ascontiguousarray(value[sl], dtype=f32),
                "hiddenT": np.ascontiguousarray(np.asarray(hidden)[sl].T, dtype=f32),
                "Wq": np.ascontiguousarray(Wq, dtype=f32),
                "bq": np.ascontiguousarray(np.asarray(bq).reshape(1, OUT), dtype=f32),
                "w_fold": np.ascontiguousarray(w_fold, dtype=f32),
                "seq_f32": np.ascontiguousarray(
                    np.asarray(seq_lens).reshape(-1)[sl].reshape(1, BL), dtype=f32
                ),
                "iota_f32": iota,
            }
        )
    return in_maps


def run(in_maps, **kw):
    from concourse.bass_utils import run_bass_kernel_spmd

    nc = get_nc()
    return run_bass_kernel_spmd(nc, in_maps, core_ids=list(range(NCORES)), **kw)


def kernel(hidden, key, value, seq_lens, Wq, bq, W1, b1, W2, b2):
    in_maps = make_in_maps(
        np.asarray(hidden), np.asarray(key), np.asarray(value),
        np.asarray(seq_lens), np.asarray(Wq), np.asarray(bq),
        np.asarray(W1), np.asarray(b1), np.asarray(W2), np.asarray(b2),
    )
    br = run(in_maps)
    ctx = np.concatenate(
        [br.results[c]["ctx_out"] for c in range(NCORES)], axis=0
    ).astype(np.float32)
    attn = np.concatenate(
        [br.results[c]["attn_out"] for c in range(NCORES)], axis=0
    ).astype(np.float32)
    return ctx, attn


# revision 13
# speedup vs baseline: 1.2982x; 1.2982x over previous
"""Trainium2 Bass kernel for Bahdanau-style additive attention.

Reference computation (eval mode):
    query  = hidden @ Wq + bq                        # [B, OUT]
    h      = tanh(key + query[:, None, :])           # [B, T, OUT]
    scores = (h @ W1 + b1) @ W2 + b2                 # [B, T, 1] -> [B, T]
    scores = where(pos < seq_lens, scores, -inf)
    attn   = softmax(scores, axis=-1)                # [B, T]
    context= einsum('bt,btd->bd', attn, value)       # [B, DV]
    return (context, attn)

Algebraic simplifications (exact, weight-only folds):
  * No nonlinearity between W1 and W2 (dropout is identity in eval), so
    scores = tanh(...) @ (W1 @ W2) + (b1 @ W2 + b2).  w = W1@W2 is a single
    [OUT] vector, folded on the host (1M-FLOP weight constant-fold).
  * The scalar c = b1@W2 + b2 shifts all scores equally; softmax is
    shift-invariant, so c (and with it b1, b2) drops out entirely.
  * Softmax runs without max-subtraction: |scores| <= sum|w| ~ 11, so
    exp() is far from fp32 overflow/underflow.  Masking multiplies the
    exp by a 0/1 mask (exact zeros, like exp(-inf)).

Sharding: pure data-parallel over batch. B=32 across 8 cores -> 4 batches
per core; small weights replicated; no collectives.

Per-core dataflow (layout: t on partitions, d on free axis):
  pass 1 (scores), per batch, per [128, 4, 1024] key block (f32->bf16
  cast during SWDGE DMA):
    DVE  : h = key + q_bcast            (tensor_tensor add, bf16 2x)
    ACT  : th = tanh(h)
    DVE  : scores[:, i] = sum_d th*w    (tensor_tensor_reduce, fused)
  softmax on the [128, 16] score tile (t = 128*i + p):
    ACT exp -> DVE mask-mult (iota vs seq_len) -> DVE row-sum ->
    PE partition-sum (lhsT=rowsums, rhs=ones) -> DVE reciprocal ->
    PE K=1 broadcast matmul -> scale.
  pass 2 (context): PE matmuls lhsT=exp[:, i] [128,1], rhs=value block
    (bf16 cast-DMA), PSUM-accumulated over 16 t-chunks, scaled by 1/s.
"""

import numpy as np

B, T, HID, OUT, DV = 32, 2048, 1024, 1024, 1024
NCORES = 8
BL = B // NCORES          # 4 local batches per core
NT = T // 128             # 16 t-chunks per batch
TCH = 4                   # t-chunks per DMA block
NBLK = NT // TCH          # 4 blocks per batch

_CACHE = {}


def _build_nc():
    import concourse.bass as bass  # noqa: F401
    import concourse.bacc as bacc
    import concourse.tile as tile
    from concourse import mybir

    f32 = mybir.dt.float32
    bf16 = mybir.dt.bfloat16
    Alu = mybir.AluOpType
    Act = mybir.ActivationFunctionType

    nc = bacc.Bacc("TRN2")

    key_d = nc.declare_dram_parameter("key", [BL, T, OUT], f32, isOutput=False)
    val_d = nc.declare_dram_parameter("value", [BL, T, DV], f32, isOutput=False)
    hT_d = nc.declare_dram_parameter("hiddenT", [HID, BL], f32, isOutput=False)
    wq_d = nc.declare_dram_parameter("Wq", [HID, OUT], f32, isOutput=False)
    bq_d = nc.declare_dram_parameter("bq", [1, OUT], f32, isOutput=False)
    w_d = nc.declare_dram_parameter("w_fold", [1, OUT], f32, isOutput=False)
    seq_d = nc.declare_dram_parameter("seq_f32", [1, BL], f32, isOutput=False)
    iota_d = nc.declare_dram_parameter("iota_f32", [128, NT], f32, isOutput=False)
    ctx_o = nc.declare_dram_parameter("ctx_out", [BL, DV], f32, isOutput=True)
    attn_o = nc.declare_dram_parameter("attn_out", [BL, T], f32, isOutput=True)

    with tile.TileContext(nc) as tc:
        with (
            tc.tile_pool(name="const", bufs=1) as const,
            tc.tile_pool(name="dramp", bufs=1, space="DRAM") as dramp,
            tc.tile_pool(name="qbp", bufs=2) as qbp,
            tc.tile_pool(name="keyp", bufs=5) as keyp,
            tc.tile_pool(name="thp", bufs=3) as thp,
            tc.tile_pool(name="valp", bufs=5) as valp,
            tc.tile_pool(name="scorep", bufs=2) as scorep,
            tc.tile_pool(name="smallp", bufs=4) as smallp,
            tc.tile_pool(name="outp", bufs=2) as outp,
            tc.tile_pool(name="cps", bufs=2, space="PSUM") as cps,
            tc.tile_pool(name="sps", bufs=1, space="PSUM") as sps,
        ):
            # ---------------- setup ----------------
            hT_sb = const.tile([128, 8, BL], f32)
            nc.sync.dma_start(
                out=hT_sb, in_=hT_d[:].rearrange("(c p) b -> p c b", p=128)
            )
            bq_sb = const.tile([1, OUT], f32)
            nc.sync.dma_start(out=bq_sb, in_=bq_d[:])
            iota_f = const.tile([128, NT], f32)
            nc.sync.dma_start(out=iota_f, in_=iota_d[:])
            ones14 = const.tile([1, BL], f32)
            nc.vector.memset(ones14, 1.0)
            ones_col = const.tile([128, 1], f32)
            nc.vector.memset(ones_col, 1.0)
            ones_row = const.tile([1, 128], f32)
            nc.vector.memset(ones_row, 1.0)

            # w broadcast to 128 partitions, cast to bf16 during DMA
            w_bc = const.tile([128, OUT], bf16)
            nc.gpsimd.dma_start(out=w_bc, in_=w_d[0:1, :].to_broadcast((128, OUT)))

            wq_sb = const.tile([128, 8, OUT], f32)
            nc.sync.dma_start(
                out=wq_sb, in_=wq_d[:].rearrange("(c p) d -> p c d", p=128)
            )

            # query projection: q[b, d] = sum_h hidden[b, h] Wq[h, d] + bq[d]
            # (PSUM tiles share the context-pass pool tags; temporally disjoint)
            q_ps = [
                cps.tile([BL, 512], f32, name=f"qps{i}", tag=f"cps{i}")
                for i in range(2)
            ]
            for half in range(2):
                cols = slice(512 * half, 512 * (half + 1))
                for c in range(8):
                    nc.tensor.matmul(
                        out=q_ps[half],
                        lhsT=hT_sb[:, c, :],
                        rhs=wq_sb[:, c, cols],
                        start=(c == 0),
                        stop=False,
                    )
                nc.tensor.matmul(
                    out=q_ps[half],
                    lhsT=ones14,
                    rhs=bq_sb[:, cols],
                    start=False,
                    stop=True,
                )
            q_sb = const.tile([BL, OUT], bf16)
            for half in range(2):
                cols = slice(512 * half, 512 * (half + 1))
                nc.vector.tensor_copy(out=q_sb[:, cols], in_=q_ps[half])

            # round-trip q through DRAM so it can be partition-broadcast by DMA
            q_dram = dramp.tile([BL, OUT], bf16)
            nc.sync.dma_start(out=q_dram, in_=q_sb)

            # ---------------- per-batch main loop ----------------
            for b in range(BL):
                qb = qbp.tile([128, OUT], bf16)
                nc.sync.dma_start(
                    out=qb, in_=q_dram[b : b + 1, :].to_broadcast((128, OUT))
                )
                len_b = smallp.tile([128, 1], f32)
                nc.sync.dma_start(
                    out=len_b, in_=seq_d[0:1, b : b + 1].to_broadcast((128, 1))
                )

                scores = scorep.tile([128, NT], f32)
                key_b = key_d[b].rearrange("(i p) d -> p i d", p=128)  # [128,16,1024]

                for blk in range(NBLK):
                    isl = slice(TCH * blk, TCH * (blk + 1))
                    kt = keyp.tile([128, TCH, OUT], bf16)
                    nc.gpsimd.dma_start(out=kt, in_=key_b[:, isl, :])  # f32->bf16
                    th = thp.tile([128, TCH, OUT], bf16)
                    for j in range(TCH):
                        # plain 2D [128, 1024] bf16 adds -> DVE 2x_1p mode
                        nc.vector.tensor_tensor(
                            out=th[:, j, :],
                            in0=kt[:, j, :],
                            in1=qb,
                            op=Alu.add,
                        )
                    nc.scalar.activation(out=th, in_=th, func=Act.Tanh)
                    for j in range(TCH):
                        i = TCH * blk + j
                        nc.vector.scalar_tensor_tensor(
                            out=kt[:, j, :],  # scratch (kt is dead after tanh)
                            in0=th[:, j, :],
                            scalar=1.0,
                            in1=w_bc,
                            op0=Alu.mult,
                            op1=Alu.mult,
                            accum_out=scores[:, i : i + 1],
                        )

                # ---- softmax over the [128, 16] score tile (t = 128i + p) ----
                e32 = smallp.tile([128, NT], f32)
                nc.scalar.activation(out=e32, in_=scores, func=Act.Exp)
                mask01 = smallp.tile([128, NT], f32)
                nc.vector.tensor_scalar(
                    out=mask01,
                    in0=iota_f,
                    scalar1=len_b,
                    scalar2=None,
                    op0=Alu.is_lt,
                )
                em = smallp.tile([128, NT], f32)
                nc.vector.tensor_tensor(out=em, in0=e32, in1=mask01, op=Alu.mult)
                ebf = smallp.tile([128, NT], bf16)
                nc.vector.tensor_copy(out=ebf, in_=em)
                s1 = smallp.tile([128, 1], f32)
                nc.vector.reduce_sum(out=s1, in_=em, axis=mybir.AxisListType.X)
                sum_ps = sps.tile([1, 1], f32, name="sum_ps", tag="sum_ps")
                nc.tensor.matmul(out=sum_ps, lhsT=s1, rhs=ones_col)
                r1 = smallp.tile([1, 1], f32)
                nc.vector.reciprocal(out=r1, in_=sum_ps)
                r_ps = sps.tile([128, 1], f32, name="r_ps", tag="r_ps")
                nc.tensor.matmul(out=r_ps, lhsT=ones_row, rhs=r1)
                r_sb = smallp.tile([128, 1], f32)
                nc.vector.tensor_copy(out=r_sb, in_=r_ps)

                # attn output
                attn_sb = outp.tile([128, NT], f32)
                nc.vector.tensor_scalar(
                    out=attn_sb, in0=em, scalar1=r_sb, scalar2=None, op0=Alu.mult
                )
                nc.sync.dma_start(
                    out=attn_o[b].rearrange("(i p) -> p i", p=128), in_=attn_sb
                )

                # ---- context: ctx[d] = (sum_t em[t] * value[t, d]) / s ----
                c_ps = [
                    cps.tile([1, 512], f32, name=f"cps{i}", tag=f"cps{i}")
                    for i in range(2)
                ]
                val_b = val_d[b].rearrange("(i p) d -> p i d", p=128)
                for blk in range(NBLK):
                    isl = slice(TCH * blk, TCH * (blk + 1))
                    vt = valp.tile([128, TCH, DV], bf16)
                    nc.gpsimd.dma_start(out=vt, in_=val_b[:, isl, :])
                    for j in range(TCH):
                        i = TCH * blk + j
                        for half in range(2):
                            cols = slice(512 * half, 512 * (half + 1))
                            nc.tensor.matmul(
                                out=c_ps[half],
                                lhsT=ebf[:, i : i + 1],
                                rhs=vt[:, j, cols],
                                start=(i == 0),
                                stop=(i == NT - 1),
                            )
                ctx_sb = outp.tile([1, DV], f32)
                for half in range(2):
                    cols = slice(512 * half, 512 * (half + 1))
                    nc.vector.tensor_scalar(
                        out=ctx_sb[:, cols],
                        in0=c_ps[half],
                        scalar1=r_sb[0:1],
                        scalar2=None,
                        op0=Alu.mult,
                    )
                nc.sync.dma_start(out=ctx_o[b : b + 1, :], in_=ctx_sb)

    nc.compile()
    return nc


def get_nc():
    if "nc" not in _CACHE:
        _CACHE["nc"] = _build_nc()
    return _CACHE["nc"]


_IOTA = (
    (np.arange(NT, dtype=np.float32)[None, :] * 128)
    + np.arange(128, dtype=np.float32)[:, None]
)


def make_in_maps(hidden, key, value, seq_lens, Wq, bq, W1, b1, W2, b2):
    f32 = np.float32
    # weight-only constant fold; b1/b2 drop out of softmax (shift invariance)
    w_fold = (np.asarray(W1, f32) @ np.asarray(W2, f32)).reshape(1, OUT)
    iota = np.ascontiguousarray(_IOTA, dtype=f32)
    in_maps = []
    for c in range(NCORES):
        sl = slice(BL * c, BL * (c + 1))
        in_maps.append(
            {
                "key": np.ascontiguousarray(key[sl], dtype=f32),
                "value": np.ascontiguousarray(value[sl], dtype=f32),
                "hiddenT": np.ascontiguousarray(np.asarray(hidden)[sl].T, dtype=f32),
                "Wq": np.ascontiguousarray(Wq, dtype=f32),
                "bq": np.ascontiguousarray(np.asarray(bq).reshape(1, OUT), dtype=f32),
                "w_fold": np.ascontiguousarray(w_fold, dtype=f32),
                "seq_f32": np.ascontiguousarray(
                    np.asarray(seq_lens).reshape(-1)[sl].reshape(1, BL), dtype=f32
                ),
                "iota_f32": iota,
            }
        )
    return in_maps


def run(in_maps, **kw):
    from concourse.bass_utils import run_bass_kernel_spmd

    nc = get_nc()
    return run_bass_kernel_spmd(nc, in_maps, core_ids=list(range(NCORES)), **kw)


def kernel(hidden, key, value, seq_lens, Wq, bq, W1, b1, W2, b2):
    in_maps = make_in_maps(
        np.asarray(hidden), np.asarray(key), np.asarray(value),
        np.asarray(seq_lens), np.asarray(Wq), np.asarray(bq),
        np.asarray(W1), np.asarray(b1), np.asarray(W2), np.asarray(b2),
    )
    br = run(in_maps)
    ctx = np.concatenate(
        [br.results[c]["ctx_out"] for c in range(NCORES)], axis=0
    ).astype(np.float32)
    attn = np.concatenate(
        [br.results[c]["attn_out"] for c in range(NCORES)], axis=0
    ).astype(np.float32)
    return ctx, attn


# revision 14
# speedup vs baseline: 1.4400x; 1.1092x over previous
"""Trainium2 Bass kernel for Bahdanau-style additive attention.

Reference computation (eval mode):
    query  = hidden @ Wq + bq                        # [B, OUT]
    h      = tanh(key + query[:, None, :])           # [B, T, OUT]
    scores = (h @ W1 + b1) @ W2 + b2                 # [B, T, 1] -> [B, T]
    scores = where(pos < seq_lens, scores, -inf)
    attn   = softmax(scores, axis=-1)                # [B, T]
    context= einsum('bt,btd->bd', attn, value)       # [B, DV]
    return (context, attn)

Algebraic simplifications (exact, weight-only folds):
  * No nonlinearity between W1 and W2 (dropout is identity in eval), so
    scores = tanh(...) @ (W1 @ W2) + (b1 @ W2 + b2).  w = W1@W2 is a single
    [OUT] vector, folded on the host (1M-FLOP weight constant-fold).
  * The scalar c = b1@W2 + b2 shifts all scores equally; softmax is
    shift-invariant, so c (and with it b1, b2) drops out entirely.
  * Softmax runs without max-subtraction: |scores| <= sum|w| ~ 11, so
    exp() is far from fp32 overflow/underflow.  Masking multiplies the
    exp by a 0/1 mask (exact zeros, like exp(-inf)).

Sharding: pure data-parallel over batch. B=32 across 8 cores -> 4 batches
per core; small weights replicated; no collectives.

Per-core dataflow (layout: t on partitions, d on free axis):
  pass 1 (scores), per batch, per [128, 4, 1024] key block (f32->bf16
  cast during SWDGE DMA):
    DVE  : h = key + q_bcast            (tensor_tensor add, bf16 2x)
    ACT  : th = tanh(h)
    DVE  : scores[:, i] = sum_d th*w    (tensor_tensor_reduce, fused)
  softmax on the [128, 16] score tile (t = 128*i + p):
    ACT exp -> DVE mask-mult (iota vs seq_len) -> DVE row-sum ->
    PE partition-sum (lhsT=rowsums, rhs=ones) -> DVE reciprocal ->
    PE K=1 broadcast matmul -> scale.
  pass 2 (context): PE matmuls lhsT=exp[:, i] [128,1], rhs=value block
    (bf16 cast-DMA), PSUM-accumulated over 16 t-chunks, scaled by 1/s.
"""

import ml_dtypes
import numpy as np

B, T, HID, OUT, DV = 32, 2048, 1024, 1024, 1024
NCORES = 8
BL = B // NCORES          # 4 local batches per core
NT = T // 128             # 16 t-chunks per batch
TCH = 4                   # t-chunks per DMA block
NBLK = NT // TCH          # 4 blocks per batch

_CACHE = {}


def _build_nc():
    import concourse.bass as bass  # noqa: F401
    import concourse.bacc as bacc
    import concourse.tile as tile
    from concourse import mybir

    f32 = mybir.dt.float32
    bf16 = mybir.dt.bfloat16
    Alu = mybir.AluOpType
    Act = mybir.ActivationFunctionType

    nc = bacc.Bacc("TRN2")

    key_d = nc.declare_dram_parameter("key", [BL, T, OUT], f32, isOutput=False)
    val_d = nc.declare_dram_parameter("value", [BL, T, DV], f32, isOutput=False)
    hT_d = nc.declare_dram_parameter("hiddenT", [HID, BL], f32, isOutput=False)
    wq_d = nc.declare_dram_parameter("Wq", [HID, OUT], bf16, isOutput=False)
    bq_d = nc.declare_dram_parameter("bq", [1, OUT], f32, isOutput=False)
    w_d = nc.declare_dram_parameter("w_fold", [1, OUT], f32, isOutput=False)
    seq_d = nc.declare_dram_parameter("seq_f32", [1, BL], f32, isOutput=False)
    iota_d = nc.declare_dram_parameter("iota_f32", [128, NT], f32, isOutput=False)
    ident_d = nc.declare_dram_parameter("ident_f32", [128, 128], f32, isOutput=False)
    ctx_o = nc.declare_dram_parameter("ctx_out", [BL, DV], f32, isOutput=True)
    attn_o = nc.declare_dram_parameter("attn_out", [BL, T], f32, isOutput=True)

    with tile.TileContext(nc) as tc:
        with (
            tc.tile_pool(name="const", bufs=1) as const,
            tc.tile_pool(name="dramp", bufs=1, space="DRAM") as dramp,
            tc.tile_pool(name="qbp", bufs=2) as qbp,
            tc.tile_pool(name="keyp", bufs=6) as keyp,
            tc.tile_pool(name="thp", bufs=3) as thp,
            tc.tile_pool(name="valp", bufs=8) as valp,
            tc.tile_pool(name="scorep", bufs=2) as scorep,
            tc.tile_pool(name="smallp", bufs=4) as smallp,
            tc.tile_pool(name="outp", bufs=2) as outp,
            tc.tile_pool(name="cps", bufs=2, space="PSUM") as cps,
            tc.tile_pool(name="sps", bufs=1, space="PSUM") as sps,
        ):
            # ---------------- setup ----------------
            hT_sb = const.tile([128, 8, BL], f32)
            nc.sync.dma_start(
                out=hT_sb, in_=hT_d[:].rearrange("(c p) b -> p c b", p=128)
            )
            bq_sb = const.tile([1, OUT], f32)
            nc.sync.dma_start(out=bq_sb, in_=bq_d[:])
            iota_f = const.tile([128, NT], f32)
            nc.sync.dma_start(out=iota_f, in_=iota_d[:])
            ones14 = const.tile([1, BL], f32)
            nc.vector.memset(ones14, 1.0)
            ones_col = const.tile([128, 1], f32)
            nc.vector.memset(ones_col, 1.0)
            ones_row = const.tile([1, 128], f32)
            nc.vector.memset(ones_row, 1.0)

            # w broadcast to 128 partitions, cast to bf16 during DMA
            w_bc = const.tile([128, OUT], bf16)
            nc.gpsimd.dma_start(out=w_bc, in_=w_d[0:1, :].to_broadcast((128, OUT)))

            ident = const.tile([128, 128], f32)
            nc.sync.dma_start(out=ident, in_=ident_d[:])
            wq_sb = const.tile([128, 8, OUT], bf16)
            nc.sync.dma_start(
                out=wq_sb, in_=wq_d[:].rearrange("(c p) d -> p c d", p=128)
            )
            hT_bf = const.tile([128, 8, BL], bf16)
            nc.vector.tensor_copy(out=hT_bf, in_=hT_sb)
            bq_bf = const.tile([1, OUT], bf16)
            nc.vector.tensor_copy(out=bq_bf, in_=bq_sb)
            ones14_bf = const.tile([1, BL], bf16)
            nc.vector.memset(ones14_bf, 1.0)

            # query projection: q[b, d] = sum_h hidden[b, h] Wq[h, d] + bq[d]
            # (PSUM tiles share the context-pass pool tags; temporally disjoint)
            q_ps = [
                cps.tile([BL, 512], f32, name=f"qps{i}", tag=f"cps{i}")
                for i in range(2)
            ]
            for half in range(2):
                cols = slice(512 * half, 512 * (half + 1))
                for c in range(8):
                    nc.tensor.matmul(
                        out=q_ps[half],
                        lhsT=hT_bf[:, c, :],
                        rhs=wq_sb[:, c, cols],
                        start=(c == 0),
                        stop=False,
                    )
                nc.tensor.matmul(
                    out=q_ps[half],
                    lhsT=ones14_bf,
                    rhs=bq_bf[:, cols],
                    start=False,
                    stop=True,
                )
            q_sb = const.tile([BL, OUT], bf16)
            for half in range(2):
                cols = slice(512 * half, 512 * (half + 1))
                nc.vector.tensor_copy(out=q_sb[:, cols], in_=q_ps[half])

            # round-trip q through DRAM so it can be partition-broadcast by DMA
            q_dram = dramp.tile([BL, OUT], bf16)
            nc.sync.dma_start(out=q_dram, in_=q_sb)

            # ---------------- per-batch main loop ----------------
            for b in range(BL):
                qb = qbp.tile([128, OUT], bf16)
                nc.sync.dma_start(
                    out=qb, in_=q_dram[b : b + 1, :].to_broadcast((128, OUT))
                )
                len_b = smallp.tile([128, 1], f32)
                nc.sync.dma_start(
                    out=len_b, in_=seq_d[0:1, b : b + 1].to_broadcast((128, 1))
                )

                scores = scorep.tile([128, NT], f32)
                key_b = key_d[b].rearrange("(i p) d -> p i d", p=128)  # [128,16,1024]

                for blk in range(NBLK):
                    isl = slice(TCH * blk, TCH * (blk + 1))
                    kt = keyp.tile([128, TCH, OUT], bf16)
                    nc.gpsimd.dma_start(out=kt, in_=key_b[:, isl, :])  # f32->bf16
                    th = thp.tile([128, TCH, OUT], bf16)
                    for j in range(TCH):
                        # plain 2D [128, 1024] bf16 adds -> DVE 2x_1p mode
                        nc.vector.tensor_tensor(
                            out=th[:, j, :],
                            in0=kt[:, j, :],
                            in1=qb,
                            op=Alu.add,
                        )
                    nc.scalar.activation(out=th, in_=th, func=Act.Tanh)
                    for j in range(TCH):
                        i = TCH * blk + j
                        nc.vector.scalar_tensor_tensor(
                            out=kt[:, j, :],  # scratch (kt is dead after tanh)
                            in0=th[:, j, :],
                            scalar=1.0,
                            in1=w_bc,
                            op0=Alu.mult,
                            op1=Alu.mult,
                            accum_out=scores[:, i : i + 1],
                        )

                # ---- softmax over the [128, 16] score tile (t = 128i + p) ----
                e32 = smallp.tile([128, NT], f32)
                nc.scalar.activation(out=e32, in_=scores, func=Act.Exp)
                mask01 = smallp.tile([128, NT], f32)
                nc.vector.tensor_scalar(
                    out=mask01,
                    in0=iota_f,
                    scalar1=len_b,
                    scalar2=None,
                    op0=Alu.is_lt,
                )
                em = smallp.tile([128, NT], f32)
                nc.vector.tensor_tensor(out=em, in0=e32, in1=mask01, op=Alu.mult)
                ebf = smallp.tile([128, NT], bf16)
                nc.vector.tensor_copy(out=ebf, in_=em)
                s1 = smallp.tile([128, 1], f32)
                nc.vector.reduce_sum(out=s1, in_=em, axis=mybir.AxisListType.X)
                sum_ps = sps.tile([1, 1], f32, name="sum_ps", tag="sum_ps")
                nc.tensor.matmul(out=sum_ps, lhsT=s1, rhs=ones_col)
                r1 = smallp.tile([1, 1], f32)
                nc.vector.reciprocal(out=r1, in_=sum_ps)
                r_ps = sps.tile([128, 1], f32, name="r_ps", tag="r_ps")
                nc.tensor.matmul(out=r_ps, lhsT=ones_row, rhs=r1)
                r_sb = smallp.tile([128, 1], f32)
                nc.vector.tensor_copy(out=r_sb, in_=r_ps)

                # attn output: transpose [128,16] -> [16,128] on PE, scale,
                # then store contiguously (512B per partition)
                tp_ps = sps.tile([NT, 128], f32, name="tp_ps", tag="tp_ps")
                nc.tensor.transpose(tp_ps, em, ident)
                attn_sb = outp.tile([NT, 128], f32)
                nc.vector.tensor_scalar(
                    out=attn_sb, in0=tp_ps, scalar1=r_sb[0:NT, :], scalar2=None,
                    op0=Alu.mult,
                )
                nc.sync.dma_start(
                    out=attn_o[b].rearrange("(i p) -> i p", p=128), in_=attn_sb
                )

                # ---- context: ctx[d] = (sum_t em[t] * value[t, d]) / s ----
                c_ps = [
                    cps.tile([1, 512], f32, name=f"cps{i}", tag=f"cps{i}")
                    for i in range(2)
                ]
                val_b = val_d[b].rearrange("(i p) d -> p i d", p=128)
                for blk in range(NBLK):
                    isl = slice(TCH * blk, TCH * (blk + 1))
                    vt = valp.tile([128, TCH, DV], bf16)
                    nc.gpsimd.dma_start(out=vt, in_=val_b[:, isl, :])
                    for j in range(TCH):
                        i = TCH * blk + j
                        for half in range(2):
                            cols = slice(512 * half, 512 * (half + 1))
                            nc.tensor.matmul(
                                out=c_ps[half],
                                lhsT=ebf[:, i : i + 1],
                                rhs=vt[:, j, cols],
                                start=(i == 0),
                                stop=(i == NT - 1),
                            )
                ctx_sb = outp.tile([1, DV], f32)
                for half in range(2):
                    cols = slice(512 * half, 512 * (half + 1))
                    nc.vector.tensor_scalar(
                        out=ctx_sb[:, cols],
                        in0=c_ps[half],
                        scalar1=r_sb[0:1],
                        scalar2=None,
                        op0=Alu.mult,
                    )
                nc.sync.dma_start(out=ctx_o[b : b + 1, :], in_=ctx_sb)

    nc.compile()
    return nc


def get_nc():
    if "nc" not in _CACHE:
        _CACHE["nc"] = _build_nc()
    return _CACHE["nc"]


_IDENT = np.eye(128, dtype=np.float32)
_IOTA = (
    (np.arange(NT, dtype=np.float32)[None, :] * 128)
    + np.arange(128, dtype=np.float32)[:, None]
)


def make_in_maps(hidden, key, value, seq_lens, Wq, bq, W1, b1, W2, b2):
    f32 = np.float32
    # weight-only constant fold; b1/b2 drop out of softmax (shift invariance)
    w_fold = (np.asarray(W1, f32) @ np.asarray(W2, f32)).reshape(1, OUT)
    iota = np.ascontiguousarray(_IOTA, dtype=f32)
    in_maps = []
    for c in range(NCORES):
        sl = slice(BL * c, BL * (c + 1))
        in_maps.append(
            {
                "key": np.ascontiguousarray(key[sl], dtype=f32),
                "value": np.ascontiguousarray(value[sl], dtype=f32),
                "hiddenT": np.ascontiguousarray(np.asarray(hidden)[sl].T, dtype=f32),
                "Wq": np.ascontiguousarray(Wq, dtype=ml_dtypes.bfloat16),
                "bq": np.ascontiguousarray(np.asarray(bq).reshape(1, OUT), dtype=f32),
                "w_fold": np.ascontiguousarray(w_fold, dtype=f32),
                "seq_f32": np.ascontiguousarray(
                    np.asarray(seq_lens).reshape(-1)[sl].reshape(1, BL), dtype=f32
                ),
                "iota_f32": iota,
                "ident_f32": _IDENT,
            }
        )
    return in_maps


def run(in_maps, **kw):
    from concourse.bass_utils import run_bass_kernel_spmd

    nc = get_nc()
    return run_bass_kernel_spmd(nc, in_maps, core_ids=list(range(NCORES)), **kw)


def kernel(hidden, key, value, seq_lens, Wq, bq, W1, b1, W2, b2):
    in_maps = make_in_maps(
        np.asarray(hidden), np.asarray(key), np.asarray(value),
        np.asarray(seq_lens), np.asarray(Wq), np.asarray(bq),
        np.asarray(W1), np.asarray(b1), np.asarray(W2), np.asarray(b2),
    )
    br = run(in_maps)
    ctx = np.concatenate(
        [br.results[c]["ctx_out"] for c in range(NCORES)], axis=0
    ).astype(np.float32)
    attn = np.concatenate(
        [br.results[c]["attn_out"] for c in range(NCORES)], axis=0
    ).astype(np.float32)
    return ctx, attn


# revision 15
# speedup vs baseline: 1.4987x; 1.0407x over previous
"""Trainium2 Bass kernel for Bahdanau-style additive attention.

Reference computation (eval mode):
    query  = hidden @ Wq + bq                        # [B, OUT]
    h      = tanh(key + query[:, None, :])           # [B, T, OUT]
    scores = (h @ W1 + b1) @ W2 + b2                 # [B, T, 1] -> [B, T]
    scores = where(pos < seq_lens, scores, -inf)
    attn   = softmax(scores, axis=-1)                # [B, T]
    context= einsum('bt,btd->bd', attn, value)       # [B, DV]
    return (context, attn)

Algebraic simplifications (exact, weight-only folds):
  * No nonlinearity between W1 and W2 (dropout is identity in eval), so
    scores = tanh(...) @ (W1 @ W2) + (b1 @ W2 + b2).  w = W1@W2 is a single
    [OUT] vector, folded on the host (1M-FLOP weight constant-fold).
  * The scalar c = b1@W2 + b2 shifts all scores equally; softmax is
    shift-invariant, so c (and with it b1, b2) drops out entirely.
  * Softmax runs without max-subtraction: |scores| <= sum|w| ~ 11, so
    exp() is far from fp32 overflow/underflow.  Masking multiplies the
    exp by a 0/1 mask (exact zeros, like exp(-inf)).

Sharding: pure data-parallel over batch. B=32 across 8 cores -> 4 batches
per core; small weights replicated; no collectives.

Per-core dataflow (layout: t on partitions, d on free axis):
  pass 1 (scores), per batch, per [128, 4, 1024] key block (f32->bf16
  cast during SWDGE DMA):
    DVE  : h = key + q_bcast            (tensor_tensor add, bf16 2x)
    ACT  : th = tanh(h)
    DVE  : scores[:, i] = sum_d th*w    (tensor_tensor_reduce, fused)
  softmax on the [128, 16] score tile (t = 128*i + p):
    ACT exp -> DVE mask-mult (iota vs seq_len) -> DVE row-sum ->
    PE partition-sum (lhsT=rowsums, rhs=ones) -> DVE reciprocal ->
    PE K=1 broadcast matmul -> scale.
  pass 2 (context): PE matmuls lhsT=exp[:, i] [128,1], rhs=value block
    (bf16 cast-DMA), PSUM-accumulated over 16 t-chunks, scaled by 1/s.
"""

import ml_dtypes
import numpy as np

B, T, HID, OUT, DV = 32, 2048, 1024, 1024, 1024
NCORES = 8
BL = B // NCORES          # 4 local batches per core
NT = T // 128             # 16 t-chunks per batch
TCH = 4                   # t-chunks per DMA block
NBLK = NT // TCH          # 4 blocks per batch

_CACHE = {}


def _build_nc():
    import concourse.bass as bass  # noqa: F401
    import concourse.bacc as bacc
    import concourse.tile as tile
    from concourse import mybir

    f32 = mybir.dt.float32
    bf16 = mybir.dt.bfloat16
    Alu = mybir.AluOpType
    Act = mybir.ActivationFunctionType

    nc = bacc.Bacc("TRN2")

    key_d = nc.declare_dram_parameter("key", [BL, T, OUT], f32, isOutput=False)
    val_d = nc.declare_dram_parameter("value", [BL, T, DV], f32, isOutput=False)
    hT_d = nc.declare_dram_parameter("hiddenT", [HID, BL], f32, isOutput=False)
    wq_d = nc.declare_dram_parameter("Wq", [HID, OUT], bf16, isOutput=False)
    bq_d = nc.declare_dram_parameter("bq", [1, OUT], f32, isOutput=False)
    w_d = nc.declare_dram_parameter("w_fold", [1, OUT], f32, isOutput=False)
    seq_d = nc.declare_dram_parameter("seq_f32", [1, BL], f32, isOutput=False)
    iota_d = nc.declare_dram_parameter("iota_f32", [128, NT], f32, isOutput=False)
    ident_d = nc.declare_dram_parameter("ident_f32", [128, 128], f32, isOutput=False)
    ctx_o = nc.declare_dram_parameter("ctx_out", [BL, DV], f32, isOutput=True)
    attn_o = nc.declare_dram_parameter("attn_out", [BL, T], f32, isOutput=True)

    with tile.TileContext(nc) as tc:
        with (
            tc.tile_pool(name="const", bufs=1) as const,
            tc.tile_pool(name="dramp", bufs=1, space="DRAM") as dramp,
            tc.tile_pool(name="qbp", bufs=2) as qbp,
            tc.tile_pool(name="keyp", bufs=6) as keyp,
            tc.tile_pool(name="thp", bufs=3) as thp,
            tc.tile_pool(name="valp", bufs=8) as valp,
            tc.tile_pool(name="scorep", bufs=2) as scorep,
            tc.tile_pool(name="smallp", bufs=4) as smallp,
            tc.tile_pool(name="outp", bufs=2) as outp,
            tc.tile_pool(name="cps", bufs=2, space="PSUM") as cps,
            tc.tile_pool(name="sps", bufs=1, space="PSUM") as sps,
        ):
            # ---------------- setup ----------------
            hT_sb = const.tile([128, 8, BL], f32)
            nc.sync.dma_start(
                out=hT_sb, in_=hT_d[:].rearrange("(c p) b -> p c b", p=128)
            )
            bq_sb = const.tile([1, OUT], f32)
            nc.sync.dma_start(out=bq_sb, in_=bq_d[:])
            iota_f = const.tile([128, NT], f32)
            nc.sync.dma_start(out=iota_f, in_=iota_d[:])
            ones14 = const.tile([1, BL], f32)
            nc.vector.memset(ones14, 1.0)
            ones_col = const.tile([128, 1], f32)
            nc.vector.memset(ones_col, 1.0)
            ones_row = const.tile([1, 128], f32)
            nc.vector.memset(ones_row, 1.0)

            # w broadcast to 128 partitions, cast to bf16 during DMA
            w_bc = const.tile([128, OUT], bf16)
            nc.gpsimd.dma_start(out=w_bc, in_=w_d[0:1, :].to_broadcast((128, OUT)))

            ident = const.tile([128, 128], f32)
            nc.sync.dma_start(out=ident, in_=ident_d[:])
            wq_sb = const.tile([128, 8, OUT], bf16)
            nc.sync.dma_start(
                out=wq_sb, in_=wq_d[:].rearrange("(c p) d -> p c d", p=128)
            )
            hT_bf = const.tile([128, 8, BL], bf16)
            nc.vector.tensor_copy(out=hT_bf, in_=hT_sb)
            bq_bf = const.tile([1, OUT], bf16)
            nc.vector.tensor_copy(out=bq_bf, in_=bq_sb)
            ones14_bf = const.tile([1, BL], bf16)
            nc.vector.memset(ones14_bf, 1.0)

            # query projection: q[b, d] = sum_h hidden[b, h] Wq[h, d] + bq[d]
            # (PSUM tiles share the context-pass pool tags; temporally disjoint)
            q_ps = [
                cps.tile([BL, 512], f32, name=f"qps{i}", tag=f"cps{i}")
                for i in range(2)
            ]
            for half in range(2):
                cols = slice(512 * half, 512 * (half + 1))
                for c in range(8):
                    nc.tensor.matmul(
                        out=q_ps[half],
                        lhsT=hT_bf[:, c, :],
                        rhs=wq_sb[:, c, cols],
                        start=(c == 0),
                        stop=False,
                    )
                nc.tensor.matmul(
                    out=q_ps[half],
                    lhsT=ones14_bf,
                    rhs=bq_bf[:, cols],
                    start=False,
                    stop=True,
                )
            q_sb = const.tile([BL, OUT], bf16)
            for half in range(2):
                cols = slice(512 * half, 512 * (half + 1))
                nc.vector.tensor_copy(out=q_sb[:, cols], in_=q_ps[half])

            # round-trip q through DRAM so it can be partition-broadcast by DMA
            q_dram = dramp.tile([BL, OUT], bf16)
            nc.sync.dma_start(out=q_dram, in_=q_sb)

            # ---------------- phase 1: scores + softmax, all batches ----------
            # All key DMAs are issued first so the value stream follows right
            # behind them; the last batch's compute then overlaps the value
            # DMA stream instead of dangling past the end of all DMA traffic.
            ebf_l, rsb_l = [], []
            for b in range(BL):
                qb = qbp.tile([128, OUT], bf16, name=f"qb{b}", tag=f"qb{b}")
                nc.sync.dma_start(
                    out=qb, in_=q_dram[b : b + 1, :].to_broadcast((128, OUT))
                )
                len_b = smallp.tile([128, 1], f32)
                nc.sync.dma_start(
                    out=len_b, in_=seq_d[0:1, b : b + 1].to_broadcast((128, 1))
                )

                scores = scorep.tile([128, NT], f32)
                key_b = key_d[b].rearrange("(i p) d -> p i d", p=128)  # [128,16,1024]

                for blk in range(NBLK):
                    isl = slice(TCH * blk, TCH * (blk + 1))
                    kt = keyp.tile([128, TCH, OUT], bf16)
                    nc.gpsimd.dma_start(out=kt, in_=key_b[:, isl, :])  # f32->bf16
                    th = thp.tile([128, TCH, OUT], bf16)
                    for j in range(TCH):
                        # plain 2D [128, 1024] bf16 adds -> DVE 2x_1p mode
                        nc.vector.tensor_tensor(
                            out=th[:, j, :],
                            in0=kt[:, j, :],
                            in1=qb,
                            op=Alu.add,
                        )
                    nc.scalar.activation(out=th, in_=th, func=Act.Tanh)
                    for j in range(TCH):
                        i = TCH * blk + j
                        nc.vector.scalar_tensor_tensor(
                            out=kt[:, j, :],  # scratch (kt is dead after tanh)
                            in0=th[:, j, :],
                            scalar=1.0,
                            in1=w_bc,
                            op0=Alu.mult,
                            op1=Alu.mult,
                            accum_out=scores[:, i : i + 1],
                        )

                # ---- softmax over the [128, 16] score tile (t = 128i + p) ----
                e32 = smallp.tile([128, NT], f32)
                nc.scalar.activation(out=e32, in_=scores, func=Act.Exp)
                mask01 = smallp.tile([128, NT], f32)
                nc.vector.tensor_scalar(
                    out=mask01,
                    in0=iota_f,
                    scalar1=len_b,
                    scalar2=None,
                    op0=Alu.is_lt,
                )
                em = smallp.tile([128, NT], f32)
                nc.vector.tensor_tensor(out=em, in0=e32, in1=mask01, op=Alu.mult)
                ebf = smallp.tile([128, NT], bf16, name=f"ebf{b}", tag=f"ebf{b}")
                nc.vector.tensor_copy(out=ebf, in_=em)
                s1 = smallp.tile([128, 1], f32)
                nc.vector.reduce_sum(out=s1, in_=em, axis=mybir.AxisListType.X)
                sum_ps = sps.tile([1, 1], f32, name="sum_ps", tag="sum_ps")
                nc.tensor.matmul(out=sum_ps, lhsT=s1, rhs=ones_col)
                r1 = smallp.tile([1, 1], f32)
                nc.vector.reciprocal(out=r1, in_=sum_ps)
                r_ps = sps.tile([128, 1], f32, name="r_ps", tag="r_ps")
                nc.tensor.matmul(out=r_ps, lhsT=ones_row, rhs=r1)
                r_sb = smallp.tile([128, 1], f32, name=f"rsb{b}", tag=f"rsb{b}")
                nc.vector.tensor_copy(out=r_sb, in_=r_ps)

                # attn output: transpose [128,16] -> [16,128] on PE, scale,
                # then store contiguously (512B per partition)
                tp_ps = sps.tile([NT, 128], f32, name="tp_ps", tag="tp_ps")
                nc.tensor.transpose(tp_ps, em, ident)
                attn_sb = outp.tile([NT, 128], f32)
                nc.vector.tensor_scalar(
                    out=attn_sb, in0=tp_ps, scalar1=r_sb[0:NT, :], scalar2=None,
                    op0=Alu.mult,
                )
                nc.sync.dma_start(
                    out=attn_o[b].rearrange("(i p) -> i p", p=128), in_=attn_sb
                )
                ebf_l.append(ebf)
                rsb_l.append(r_sb)

            # ---------------- phase 2: context, all batches --------------
            for b in range(BL):
                ebf, r_sb = ebf_l[b], rsb_l[b]
                c_ps = [
                    cps.tile([1, 512], f32, name=f"cps{i}", tag=f"cps{i}")
                    for i in range(2)
                ]
                val_b = val_d[b].rearrange("(i p) d -> p i d", p=128)
                for blk in range(NBLK):
                    isl = slice(TCH * blk, TCH * (blk + 1))
                    vt = valp.tile([128, TCH, DV], bf16)
                    nc.gpsimd.dma_start(out=vt, in_=val_b[:, isl, :])
                    for j in range(TCH):
                        i = TCH * blk + j
                        for half in range(2):
                            cols = slice(512 * half, 512 * (half + 1))
                            nc.tensor.matmul(
                                out=c_ps[half],
                                lhsT=ebf[:, i : i + 1],
                                rhs=vt[:, j, cols],
                                start=(i == 0),
                                stop=(i == NT - 1),
                            )
                ctx_sb = outp.tile([1, DV], f32)
                for half in range(2):
                    cols = slice(512 * half, 512 * (half + 1))
                    nc.vector.tensor_scalar(
                        out=ctx_sb[:, cols],
                        in0=c_ps[half],
                        scalar1=r_sb[0:1],
                        scalar2=None,
                        op0=Alu.mult,
                    )
                nc.sync.dma_start(out=ctx_o[b : b + 1, :], in_=ctx_sb)

    nc.compile()
    return nc


def get_nc():
    if "nc" not in _CACHE:
        _CACHE["nc"] = _build_nc()
    return _CACHE["nc"]


_IDENT = np.eye(128, dtype=np.float32)
_IOTA = (
    (np.arange(NT, dtype=np.float32)[None, :] * 128)
    + np.arange(128, dtype=np.float32)[:, None]
)


def make_in_maps(hidden, key, value, seq_lens, Wq, bq, W1, b1, W2, b2):
    f32 = np.float32
    # weight-only constant fold; b1/b2 drop out of softmax (shift invariance)
    w_fold = (np.asarray(W1, f32) @ np.asarray(W2, f32)).reshape(1, OUT)
    iota = np.ascontiguousarray(_IOTA, dtype=f32)
    in_maps = []
    for c in range(NCORES):
        sl = slice(BL * c, BL * (c + 1))
        in_maps.append(
            {
                "key": np.ascontiguousarray(key[sl], dtype=f32),
                "value": np.ascontiguousarray(value[sl], dtype=f32),
                "hiddenT": np.ascontiguousarray(np.asarray(hidden)[sl].T, dtype=f32),
                "Wq": np.ascontiguousarray(Wq, dtype=ml_dtypes.bfloat16),
                "bq": np.ascontiguousarray(np.asarray(bq).reshape(1, OUT), dtype=f32),
                "w_fold": np.ascontiguousarray(w_fold, dtype=f32),
                "seq_f32": np.ascontiguousarray(
                    np.asarray(seq_lens).reshape(-1)[sl].reshape(1, BL), dtype=f32
                ),
                "iota_f32": iota,
                "ident_f32": _IDENT,
            }
        )
    return in_maps


def run(in_maps, **kw):
    from concourse.bass_utils import run_bass_kernel_spmd

    nc = get_nc()
    return run_bass_kernel_spmd(nc, in_maps, core_ids=list(range(NCORES)), **kw)


def kernel(hidden, key, value, seq_lens, Wq, bq, W1, b1, W2, b2):
    in_maps = make_in_maps(
        np.asarray(hidden), np.asarray(key), np.asarray(value),
        np.asarray(seq_lens), np.asarray(Wq), np.asarray(bq),
        np.asarray(W1), np.asarray(b1), np.asarray(W2), np.asarray(b2),
    )
    br = run(in_maps)
    ctx = np.concatenate(
        [br.results[c]["ctx_out"] for c in range(NCORES)], axis=0
    ).astype(np.float32)
    attn = np.concatenate(
        [br.results[c]["attn_out"] for c in range(NCORES)], axis=0
    ).astype(np.float32)
    return ctx, attn
